# revision 1
# baseline (speedup 1.0000x reference)
"""Self-contained Trainium2 kernel for a dense transformer block.

Contract: kernel(**inputs) takes the FULL fp32 inputs of reference.setup_inputs()
and returns the FULL [2, 2048, 1024] fp32 output, distributing across 8
NeuronCores internally (token-sharded LN/proj/FFN + head-sharded attention,
one AllGather + one AllToAll).
"""

import numpy as np
import ml_dtypes

# ---- problem constants (hardcoded per contract) ----
B, T, D = 2, 2048, 1024
NH, DK = 16, 64
DFF = 4096
LN_EPS = 1e-5
NC_ = 8                 # cores
TS = 512                # tokens per core
P = 128                 # partitions
FC = D // P             # 8 feature chunks
M1 = DFF // P           # 32 dff tiles
NQT = 4                 # 512-token q tiles per batch
SCALE = 1.0 / np.sqrt(DK)

F32 = None
BF16 = None


def build(nc, tile, mybir, bass, solo=False):
    """Emit the SPMD per-core program into `nc` via TileContext."""
    global F32, BF16
    F32 = mybir.dt.float32
    BF16 = mybir.dt.bfloat16

    # ---- DRAM I/O ----
    x_d = nc.dram_tensor("x_sl", [P, FC, TS], F32, kind="ExternalInput").ap()
    xb_d = nc.dram_tensor("x_bf", [P, FC, TS], BF16, kind="ExternalInput").ap()
    wqk_d = nc.dram_tensor("wqk", [P, FC, 256], BF16, kind="ExternalInput").ap()
    wv_d = nc.dram_tensor("wv", [P, FC, 128], BF16, kind="ExternalInput").ap()
    wproj_d = nc.dram_tensor("wproj", [P, FC, FC, P], BF16, kind="ExternalInput").ap()
    w1_d = nc.dram_tensor("w1", [M1, P, FC, P], BF16, kind="ExternalInput").ap()
    w2_d = nc.dram_tensor("w2", [FC, P, M1, P], BF16, kind="ExternalInput").ap()
    out_d = nc.dram_tensor("out_sl", [FC, P, TS], F32, kind="ExternalOutput").ap()

    Exp = mybir.ActivationFunctionType.Exp
    Gelu = mybir.ActivationFunctionType.Gelu
    Square = mybir.ActivationFunctionType.Square
    Sqrt = mybir.ActivationFunctionType.Sqrt

    with tile.TileContext(nc) as tc:
        import contextlib
        es = contextlib.ExitStack()
        with es:
            const = es.enter_context(tc.tile_pool(name="const", bufs=1))
            persist = es.enter_context(tc.tile_pool(name="persist", bufs=1))
            dram = es.enter_context(tc.tile_pool(name="dram", bufs=1, space="DRAM"))
            work = es.enter_context(tc.tile_pool(name="work", bufs=1))

            # ---- constants ----
            ones_bf = const.tile([P, 1], BF16)
            nc.gpsimd.memset(ones_bf[:], 1.0)
            masks = []
            for d_i in range(4):
                m_t = const.tile([P, TS], F32, name=f"mask{d_i}")
                nc.gpsimd.memset(m_t[:], 0.0)
                # S^T tile [k-part, q-free], k0 = q0 + 128*d_i:
                # keep (mask 0) where q >= k i.e. qf >= kp + 128*d_i
                nc.gpsimd.affine_select(
                    out=m_t[:], in_=m_t[:],
                    compare_op=mybir.AluOpType.is_ge,
                    fill=-1e30, base=-128 * d_i,
                    pattern=[[1, TS]], channel_multiplier=-1,
                )
                masks.append(m_t)

            wqk = const.tile([P, FC, 256], BF16)
            wv = const.tile([P, FC, 128], BF16)
            wproj = const.tile([P, FC, FC, P], BF16)

            # persistent activations
            x_fm = persist.tile([P, FC, TS], F32)       # x^T fp32 (residual)
            r1 = persist.tile([P, FC, TS], F32)         # x + attnproj, fp32

            # collective DRAM buffers (AG split by feature half, A2A by head)
            HD = D // 2
            ag_in = [dram.tile([HD, TS], BF16, name=f"ag_in{i}") for i in range(2)]
            ag_out = [dram.tile([NC_, HD, TS], BF16, name=f"ag_out{i}",
                                addr_space="Local" if solo else "Shared")
                      for i in range(2)]
            a2a_in = [dram.tile([NC_, 64, TS], BF16, name=f"a2a_in{i}")
                      for i in range(2)]
            a2a_out = [dram.tile([NC_, 64, TS], BF16, name=f"a2a_out{i}")
                       for i in range(2)]

            # =============== Stage A: load x, transpose, LN1 ===============
            with tc.tile_pool(name="poolA", bufs=1) as poolA, \
                 tc.tile_pool(name="psumA", bufs=2, space="PSUM") as psum:
                x_bf = poolA.tile([P, FC, TS], BF16)
                for fc in range(FC):
                    nc.sync.dma_start(x_bf[:, fc, :], xb_d[:, fc, :])
                # fp32 x for the residual: off the LN critical path
                nc.sync.dma_start(x_fm[:], x_d[:])

                aT = poolA.tile([P, FC, TS], BF16)
                _emit_ln(nc, tc, psum, work, mybir, x_bf, aT, ones_bf)

                # Stage B: AllGather of aT, split into two feature halves so
                # qkv accumulation can start after the first half lands
                for i in range(2):
                    nc.sync.dma_start(
                        ag_in[i][:].rearrange("(fc p) t -> p fc t", p=P),
                        aT[:, 4 * i:4 * i + 4, :])
                    if solo:
                        for s in range(NC_):
                            nc.sync.dma_start(ag_out[i][s], ag_in[i][:])
                    else:
                        nc.gpsimd.collective_compute(
                            "AllGather", mybir.AluOpType.bypass,
                            replica_groups=[list(range(NC_))],
                            ins=[ag_in[i].opt()], outs=[ag_out[i].opt()],
                        )

            # =============== Stage C: qkv for local 2 heads ===============
            nc.sync.dma_start(wqk[:], wqk_d[:])
            nc.sync.dma_start(wv[:], wv_d[:])
            with tc.tile_pool(name="poolC", bufs=1) as poolC, \
                 tc.tile_pool(name="agpool", bufs=4) as agpool:
                qT = poolC.tile([P, NC_, TS], BF16)
                kT = poolC.tile([P, NC_, TS], BF16)
                # v~ layout per head block of 128 cols: col 0 = ones,
                # cols 1:64 = zeros, cols 64:128 = v  (PV psum: row 0 =
                # sum(exp), rows 64:128 = V^T P^T; 64-partition slices must
                # start at 0 or 64)
                v_sb = poolC.tile([P, 32, 256], BF16)
                nc.gpsimd.memset(v_sb[:, :, 0:1], 1.0)
                nc.gpsimd.memset(v_sb[:, :, 1:64], 0.0)
                nc.gpsimd.memset(v_sb[:, :, 128:129], 1.0)
                nc.gpsimd.memset(v_sb[:, :, 129:192], 0.0)

                psumCD_cm = tc.tile_pool(name="psumCD", bufs=2, space="PSUM")
                psum = psumCD_cm.__enter__()
                # per-head outputs at partitions 64..128 so the DVE normalize
                # keeps in/out partition ranges identical and 64-aligned
                oT_h = [poolC.tile([P, NC_, TS], BF16, name=f"oT{h}")
                        for h in range(2)]

                def emit_qkv(cb):
                    ag_sb = agpool.tile([P, FC, TS], BF16, tag="ag_sb")
                    for i in range(2):
                        nc.sync.dma_start(
                            ag_sb[:, 4 * i:4 * i + 4, :],
                            ag_out[i][cb].rearrange("(fc p) t -> p fc t", p=P))
                    ps_q = psum.tile([P, TS], F32, tag="psqk", bufs=2)
                    for fc in range(FC):
                        nc.tensor.matmul(ps_q[:], wqk[:, fc, 0:128], ag_sb[:, fc, :],
                                         start=(fc == 0), stop=(fc == FC - 1))
                    nc.scalar.copy(qT[:, cb, :], ps_q[:])
                    ps_k = psum.tile([P, TS], F32, tag="psqk", bufs=2)
                    for fc in range(FC):
                        nc.tensor.matmul(ps_k[:], wqk[:, fc, 128:256], ag_sb[:, fc, :],
                                         start=(fc == 0), stop=(fc == FC - 1))
                    nc.scalar.copy(kT[:, cb, :], ps_k[:])
                    for st in range(4):
                        ps_v = psum.tile([P, P], F32, tag="psv", bufs=2)
                        for fc in range(FC):
                            nc.tensor.matmul(
                                ps_v[:], ag_sb[:, fc, st * P:(st + 1) * P],
                                wv[:, fc, :],
                                start=(fc == 0), stop=(fc == FC - 1))
                        tt = cb * 4 + st
                        nc.vector.tensor_copy(v_sb[:, tt, 64:128], ps_v[:, 0:64])
                        nc.vector.tensor_copy(v_sb[:, tt, 192:256], ps_v[:, 64:128])

                def emit_attn_qtile(h, b, qt):
                    hr = slice(h * 64, (h + 1) * 64)
                    ps_o = psum.tile([P, TS], F32, tag="pso", bufs=2)
                    nkc = 4 * qt + 4
                    for pr in range(nkc // 2):
                        # two k-chunks share one psum pair + one exp
                        ps_s = psum.tile([P, 2, TS], F32, tag="pss", bufs=3)
                        for j in range(2):
                            kc = 2 * pr + j
                            cb_k = 4 * b + kc // 4
                            sl = (kc % 4) * P
                            nc.tensor.matmul(
                                ps_s[:, j, :],
                                kT[hr, cb_k, sl:sl + P],
                                qT[hr, 4 * b + qt, :],
                                start=True, stop=True)
                            if kc >= 4 * qt:
                                nc.vector.tensor_add(
                                    ps_s[:, j, :], ps_s[:, j, :],
                                    masks[kc - 4 * qt][:])
                        pT = work.tile([P, 2, TS], BF16, tag="pT", bufs=3)
                        nc.scalar.activation(pT[:], ps_s[:], Exp)
                        for j in range(2):
                            kc = 2 * pr + j
                            # ps_o row 0 = sum(exp), rows 64:128 = V^T P^T
                            nc.tensor.matmul(
                                ps_o[:],
                                v_sb[:, 16 * b + kc, h * P:(h + 1) * P],
                                pT[:, j, :],
                                start=(kc == 0), stop=(kc == nkc - 1))
                    rec = work.tile([1, TS], F32, tag="rec", bufs=2)
                    nc.vector.reciprocal(rec[:], ps_o[0:1, :])
                    recb = work.tile([P, TS], F32, tag="recb", bufs=2)
                    nc.gpsimd.partition_broadcast(recb[:], rec[:])
                    nc.vector.tensor_mul(
                        oT_h[h][64:128, 4 * b + qt, :],
                        ps_o[64:128, :], recb[64:128, :])
                    nc.sync.dma_start(
                        a2a_in[h][4 * b + qt].rearrange("p t -> p t"),
                        oT_h[h][64:128, 4 * b + qt, :])

                def emit_a2a(h):
                    if solo:
                        for s in range(NC_):
                            nc.sync.dma_start(a2a_out[h][s], a2a_in[h][s])
                    else:
                        nc.gpsimd.collective_compute(
                            "AllToAll", mybir.AluOpType.bypass,
                            replica_groups=[list(range(NC_))],
                            ins=[a2a_in[h].opt()], outs=[a2a_out[h].opt()],
                        )

                # sequential: all qkv, then attention (separate psum scopes)
                for cb in range(NC_):
                    emit_qkv(cb)
                psumCD_cm.__exit__(None, None, None)
                psumD_cm = tc.tile_pool(name="psumD", bufs=2, space="PSUM")
                psum = psumD_cm.__enter__()
                nc.sync.dma_start(wproj[:], wproj_d[:])
                for h in range(2):
                    for b in range(2):
                        for qt in reversed(range(NQT)):
                            emit_attn_qtile(h, b, qt)
                    emit_a2a(h)

                psumD_cm.__exit__(None, None, None)

            # =============== Stage F: proj + residual + LN2 ===============
            with tc.tile_pool(name="poolF", bufs=1) as poolF:
                psumF_cm = tc.tile_pool(name="psumF", bufs=2, space="PSUM")
                psum = psumF_cm.__enter__()
                attn_fm = poolF.tile([P, NC_, TS], BF16)
                for h in range(2):
                    nc.sync.dma_start(
                        attn_fm[h * 64:(h + 1) * 64, :, :],
                        a2a_out[h][:].rearrange("s p t -> p s t"))
                r1_bf = poolF.tile([P, FC, TS], BF16)
                for m in range(FC):
                    ps_p = psum.tile([P, TS], F32, tag="psp", bufs=3)
                    for s in range(NC_):
                        nc.tensor.matmul(ps_p[:], wproj[:, m, s, :],
                                         attn_fm[:, s, :],
                                         start=(s == 0), stop=(s == NC_ - 1))
                    nc.vector.tensor_add(r1[:, m, :], ps_p[:], x_fm[:, m, :])
                    nc.scalar.copy(r1_bf[:, m, :], r1[:, m, :])

                bT = poolF.tile([P, FC, TS], BF16)
                _emit_ln(nc, tc, psum, work, mybir, r1_bf, bT, ones_bf)
                psumF_cm.__exit__(None, None, None)

                # =============== Stage G: FFN ===============
                with tc.tile_pool(name="hpool", bufs=1) as hpool, \
                     tc.tile_pool(name="w1pool", bufs=4) as w1pool, \
                     tc.tile_pool(name="w2pool", bufs=3) as w2pool, \
                     tc.tile_pool(name="psumG", bufs=2, space="PSUM") as psumG:
                    hT = hpool.tile([P, M1, TS], BF16)
                    for j in range(M1 // 2):
                        # two m1 tiles share one psum pair + one gelu
                        w1_t = w1pool.tile([P, 2, FC, P], BF16, tag="w1t")
                        nc.sync.dma_start(
                            w1_t[:],
                            w1_d[2 * j:2 * j + 2].rearrange("m p fc c -> p m fc c"))
                        ps_h = psumG.tile([P, 2, TS], F32, tag="psh")
                        for half in range(2):
                            for fc in range(FC):
                                nc.tensor.matmul(
                                    ps_h[:, half, :], w1_t[:, half, fc, :],
                                    bT[:, fc, :],
                                    start=(fc == 0), stop=(fc == FC - 1))
                        nc.scalar.activation(
                            hT[:, 2 * j:2 * j + 2, :], ps_h[:], Gelu)

                    for m2 in range(FC):
                        w2_t = w2pool.tile([P, M1, P], BF16, tag="w2t")
                        nc.sync.dma_start(w2_t[:], w2_d[m2])
                        ps_f = psumG.tile([P, TS], F32, tag="psf")
                        for kc in range(M1):
                            nc.tensor.matmul(ps_f[:], w2_t[:, kc, :], hT[:, kc, :],
                                             start=(kc == 0), stop=(kc == M1 - 1))
                        of = work.tile([P, TS], F32, tag="of", bufs=2)
                        nc.vector.tensor_add(of[:], ps_f[:], r1[:, m2, :])
                        nc.sync.dma_start(out_d[m2], of[:])
    return nc


def _emit_ln(nc, tc, psum, work, mybir, x_bf, out_bf, ones_bf):
    """LayerNorm over features (partition axis spread over FC chunks),
    feature-major layout. out = (x - mu) * rsqrt(var + eps), bf16.
    Gains/biases are folded into downstream weights on the host."""
    F32 = mybir.dt.float32
    BF16 = mybir.dt.bfloat16
    Square = mybir.ActivationFunctionType.Square
    Sqrt = mybir.ActivationFunctionType.Sqrt

    eps_t = work.tile([1, 1], F32, tag="eps")
    nc.gpsimd.memset(eps_t[:], LN_EPS)
    ps_sum = psum.tile([1, TS], F32, tag="st1", bufs=1)
    ps_sq = psum.tile([1, TS], F32, tag="st2", bufs=1)
    for fc in range(FC):
        sq = work.tile([P, TS], BF16, tag="sq", bufs=2)
        nc.scalar.activation(sq[:], x_bf[:, fc, :], Square)
        nc.tensor.matmul(ps_sum[:], ones_bf[:], x_bf[:, fc, :],
                         start=(fc == 0), stop=(fc == FC - 1))
        nc.tensor.matmul(ps_sq[:], ones_bf[:], sq[:],
                         start=(fc == 0), stop=(fc == FC - 1))
    mu = work.tile([1, TS], F32, tag="mu")
    nc.scalar.mul(mu[:], ps_sum[:], 1.0 / D)
    msq = work.tile([1, TS], F32, tag="msq")
    nc.scalar.mul(msq[:], ps_sq[:], 1.0 / D)
    mu2 = work.tile([1, TS], F32, tag="mu2")
    nc.vector.tensor_mul(mu2[:], mu[:], mu[:])
    var = work.tile([1, TS], F32, tag="var")
    nc.vector.tensor_sub(var[:], msq[:], mu2[:])
    sd = work.tile([1, TS], F32, tag="sd")
    nc.scalar.activation(sd[:], var[:], Sqrt, bias=eps_t[:])
    n1 = work.tile([1, TS], F32, tag="n1")
    nc.vector.reciprocal(n1[:], sd[:])
    n2 = work.tile([1, TS], F32, tag="n2")
    nc.vector.scalar_tensor_tensor(
        out=n2[:], in0=mu[:], scalar=-1.0, in1=n1[:],
        op0=mybir.AluOpType.mult, op1=mybir.AluOpType.mult)
    n1b = work.tile([P, TS], F32, tag="n1b")
    nc.gpsimd.partition_broadcast(n1b[:], n1[:])
    n2b = work.tile([P, TS], F32, tag="n2b")
    nc.gpsimd.partition_broadcast(n2b[:], n2[:])
    for fc in range(FC):
        t = work.tile([P, TS], F32, tag="lnt", bufs=2)
        nc.vector.tensor_mul(t[:], x_bf[:, fc, :], n1b[:])
        nc.vector.tensor_add(out_bf[:, fc, :], t[:], n2b[:])


# ==================== host side ====================

_CACHE = {}


def _build_and_compile():
    if "nc" in _CACHE:
        return _CACHE["nc"]
    import concourse.bass as bass
    import concourse.mybir as mybir
    import concourse.tile as tile
    from concourse import bacc
    nc = bacc.Bacc("TRN2", target_bir_lowering=False, debug=False,
                   num_devices=NC_)
    build(nc, tile, mybir, bass, solo=False)
    nc.compile()
    _CACHE["nc"] = nc
    return nc


def _prep_inputs(x, w_qkv, w_proj, w1, w2, ln1_g, ln1_b, ln2_g, ln2_b):
    bf = ml_dtypes.bfloat16
    x = np.asarray(x, np.float32)
    w_qkv = np.asarray(w_qkv, np.float32)
    w_proj = np.asarray(w_proj, np.float32)
    w1 = np.asarray(w1, np.float32)
    w2 = np.asarray(w2, np.float32)
    ln1_g = np.asarray(ln1_g, np.float32)
    ln2_g = np.asarray(ln2_g, np.float32)
    assert not np.any(np.asarray(ln1_b)) and not np.any(np.asarray(ln2_b)), \
        "nonzero LN bias not wired in this build"

    x_flat = np.ascontiguousarray(x.reshape(B * T, D))
    wq = w_qkv[:, :D] * (SCALE * ln1_g[:, None])
    wk = w_qkv[:, D:2 * D] * ln1_g[:, None]
    wv_full = w_qkv[:, 2 * D:] * ln1_g[:, None]
    w1f = w1 * ln2_g[:, None]

    # [m1, p, fc, c] layouts
    w1_t = np.ascontiguousarray(
        w1f.reshape(FC, P, M1, P).transpose(2, 1, 0, 3)).astype(bf)
    w2_t = np.ascontiguousarray(
        w2.reshape(M1, P, FC, P).transpose(2, 1, 0, 3)).astype(bf)
    wproj_t = np.ascontiguousarray(
        w_proj.reshape(FC, P, FC, P).transpose(1, 2, 0, 3)).astype(bf)

    in_maps = []
    for c in range(NC_):
        hcols = slice(2 * c * DK, 2 * c * DK + 128)
        wqk_c = np.concatenate([wq[:, hcols], wk[:, hcols]], axis=1)  # [1024, 256]
        wqk_t = np.ascontiguousarray(
            wqk_c.reshape(FC, P, 256).transpose(1, 0, 2)).astype(bf)
        wv_t = np.ascontiguousarray(
            wv_full[:, hcols].reshape(FC, P, P).transpose(1, 0, 2)).astype(bf)
        x_c = x_flat[c * TS:(c + 1) * TS]          # [TS, D]
        x_cT = np.ascontiguousarray(
            x_c.T.reshape(FC, P, TS).transpose(1, 0, 2))  # [P, FC, TS]
        in_maps.append({
            "x_sl": x_cT,
            "x_bf": x_cT.astype(ml_dtypes.bfloat16),
            "wqk": wqk_t,
            "wv": wv_t,
            "wproj": wproj_t,
            "w1": w1_t,
            "w2": w2_t,
        })
    return in_maps


def kernel(x, w_qkv, w_proj, w1, w2, ln1_g, ln1_b, ln2_g, ln2_b):
    from concourse.bass_utils import run_bass_kernel_spmd
    nc = _build_and_compile()
    in_maps = _prep_inputs(x, w_qkv, w_proj, w1, w2,
                           ln1_g, ln1_b, ln2_g, ln2_b)
    res = run_bass_kernel_spmd(nc, in_maps, list(range(NC_)))
    slices = []
    for c in range(NC_):
        o = res.results[c]["out_sl"]            # [FC, P, TS]
        slices.append(o.transpose(2, 0, 1).reshape(TS, D))
    out = np.concatenate(slices, axis=0)
    return np.ascontiguousarray(out.reshape(B, T, D)).astype(np.float32)



# revision 11
# speedup vs baseline: 1.7306x; 1.7306x over previous
"""Self-contained Trainium2 kernel for a dense transformer block.

Contract: kernel(**inputs) takes the FULL fp32 inputs of reference.setup_inputs()
and returns the FULL [2, 2048, 1024] fp32 output, distributing across 8
NeuronCores internally (token-sharded LN/proj/FFN + head-sharded attention,
one AllGather + one AllToAll).

v2: fp8(e4m3, x64-scaled weights) DoubleRow matmuls for qkv/v/PV/proj/ffn,
bf16 scores with exp-side dequant, causal extent trimming, engine-balanced
elementwise work, pipelined per-pair AllGather.
"""

import numpy as np
import ml_dtypes

# ---- problem constants (hardcoded per contract) ----
B, T, D = 2, 2048, 1024
NH, DK = 16, 64
DFF = 4096
LN_EPS = 1e-5
NC_ = 8                 # cores
TS = 512                # tokens per core
P = 128                 # partitions
FC = D // P             # 8 feature chunks
FCP = FC // 2           # 4 fc pairs (DoubleRow k-tile pairs)
M1 = DFF // P           # 32 dff tiles
M1P = M1 // 2           # 16 dff tile pairs
NQT = 4                 # 512-token q tiles per batch
SCALE = 1.0 / np.sqrt(DK)
WS = 64.0               # fp8 weight scale

F32 = None
BF16 = None
FP8 = None


def build(nc, tile, mybir, bass, solo=False):
    """Emit the SPMD per-core program into `nc` via TileContext."""
    global F32, BF16, FP8
    F32 = mybir.dt.float32
    BF16 = mybir.dt.bfloat16
    FP8 = mybir.dt.float8e4
    DR = mybir.MatmulPerfMode.DoubleRow

    # ---- DRAM I/O ----
    x_d = nc.dram_tensor("x_sl", [P, FC, TS], F32, kind="ExternalInput").ap()
    xb_d = nc.dram_tensor("x_bf", [P, FC, TS], BF16, kind="ExternalInput").ap()
    wqk_d = nc.dram_tensor("wqk", [P, FCP, 2, 256], FP8, kind="ExternalInput").ap()
    wv_d = nc.dram_tensor("wv", [P, FCP, 2, 128], FP8, kind="ExternalInput").ap()
    wproj_d = nc.dram_tensor("wproj", [P, 4, 2, FC, P], FP8, kind="ExternalInput").ap()
    w1_d = nc.dram_tensor("w1", [M1, P, FCP, 2, P], FP8, kind="ExternalInput").ap()
    w2_d = nc.dram_tensor("w2", [FC, P, M1P, 2, P], FP8, kind="ExternalInput").ap()
    out_d = nc.dram_tensor("out_sl", [FC, P, TS], F32, kind="ExternalOutput").ap()

    Exp = mybir.ActivationFunctionType.Exp
    Gelu = mybir.ActivationFunctionType.Gelu
    Ln = mybir.ActivationFunctionType.Ln

    with tile.TileContext(nc) as tc:
        import contextlib
        es = contextlib.ExitStack()
        with es:
            const = es.enter_context(tc.tile_pool(name="const", bufs=1))
            persist = es.enter_context(tc.tile_pool(name="persist", bufs=1))
            dram = es.enter_context(tc.tile_pool(name="dram", bufs=1, space="DRAM"))
            work = es.enter_context(tc.tile_pool(name="work", bufs=1))

            # ---- constants ----
            ones_bf = const.tile([P, 1], BF16)
            nc.gpsimd.memset(ones_bf[:], 1.0)
            eps_t = const.tile([1, 1], F32)
            nc.gpsimd.memset(eps_t[:], LN_EPS)

            wqk = const.tile([P, FCP, 2, 256], FP8)
            wv = const.tile([P, FCP, 2, 128], FP8)
            wproj = const.tile([P, 4, 2, FC, P], FP8)

            # persistent activations
            x_fm = persist.tile([P, FC, TS], F32)       # x^T fp32 (residual)
            r1 = persist.tile([P, FC, TS], F32)         # x + attnproj, fp32
            r1_bf = persist.tile([P, FC, TS], BF16)
            qkT = persist.tile([P, 2, NC_, TS], BF16)   # [.,0,..]=q [.,1,..]=k
            # v~ per (b, kpair, i): 256 cols = 2 head blocks of
            # [ones(1) zeros(63) v(64)]; holds WS*v
            v_sb = persist.tile([P, B, 8, 2, 256], FP8)
            oT = [persist.tile([P, NC_, TS], FP8, name=f"oT{h}") for h in range(2)]
            attn_fm = persist.tile([P, NC_, TS], FP8)
            bT = persist.tile([P, FCP, 2, TS], FP8)
            hT = persist.tile([P, M1P, 2, TS], FP8)

            # collective DRAM buffers
            ag_in = [dram.tile([2, P, TS], FP8, name=f"ag_in{p}") for p in range(FCP)]
            ag_out = [dram.tile([NC_, 2, P, TS], FP8, name=f"ag_out{p}",
                                addr_space="Local" if solo else "Shared")
                      for p in range(FCP)]
            a2a_in = [dram.tile([NC_, 64, TS], FP8, name=f"a2a_in{h}")
                      for h in range(2)]
            a2a_out = [dram.tile([NC_, 64, TS], FP8, name=f"a2a_out{h}")
                       for h in range(2)]

            # v~ constant columns
            for h in range(2):
                nc.gpsimd.memset(v_sb[:, :, :, :, 128 * h:128 * h + 1], 1.0)
                nc.gpsimd.memset(v_sb[:, :, :, :, 128 * h + 1:128 * h + 64], 0.0)

            def emit_ln(psum, x_src, out_dst, tag):
                """LN stats+normalize: x_src [P,FC,TS] bf16 ->
                out_dst [P,FCP,2,TS] fp8 (gains folded in weights)."""
                ps_sum = psum.tile([1, TS], F32, tag=f"{tag}s", bufs=1)
                ps_sq = psum.tile([1, TS], F32, tag=f"{tag}q", bufs=1)
                for fc in range(FC):
                    sq = work.tile([P, TS], BF16, tag="sq", bufs=2)
                    nc.vector.tensor_mul(sq[:], x_src[:, fc, :], x_src[:, fc, :])
                    nc.tensor.matmul(ps_sum[:], ones_bf[:], x_src[:, fc, :],
                                     start=(fc == 0), stop=(fc == FC - 1))
                    nc.tensor.matmul(ps_sq[:], ones_bf[:], sq[:],
                                     start=(fc == 0), stop=(fc == FC - 1))
                mu = work.tile([1, TS], F32, tag="mu")
                nc.vector.tensor_scalar_mul(mu[:], ps_sum[:], 1.0 / D)
                msq = work.tile([1, TS], F32, tag="msq")
                nc.vector.tensor_scalar_mul(msq[:], ps_sq[:], 1.0 / D)
                mu2 = work.tile([1, TS], F32, tag="mu2")
                nc.vector.tensor_mul(mu2[:], mu[:], mu[:])
                var = work.tile([1, TS], F32, tag="var")
                nc.vector.tensor_sub(var[:], msq[:], mu2[:])
                # rsqrt via ln/exp (keeps Act on the ln+exp table set)
                lnv = work.tile([1, TS], F32, tag="lnv")
                nc.scalar.activation(lnv[:], var[:], Ln, bias=eps_t[:])
                n1_bf = work.tile([1, TS], BF16, tag="n1bf")
                nc.scalar.activation(n1_bf[:], lnv[:], Exp, scale=-0.5)
                n2_bf = work.tile([1, TS], BF16, tag="n2bf")
                nc.vector.scalar_tensor_tensor(
                    out=n2_bf[:], in0=mu[:], scalar=-1.0, in1=n1_bf[:],
                    op0=mybir.AluOpType.mult, op1=mybir.AluOpType.mult)
                n1b = work.tile([P, TS], BF16, tag="n1b")
                nc.gpsimd.partition_broadcast(n1b[:], n1_bf[:])
                n2b = work.tile([P, TS], BF16, tag="n2b")
                nc.gpsimd.partition_broadcast(n2b[:], n2_bf[:])
                for fc in range(FC):
                    t = work.tile([P, TS], BF16, tag="lnt", bufs=3)
                    nc.vector.tensor_mul(t[:], x_src[:, fc, :], n1b[:])
                    nc.gpsimd.tensor_add(out_dst[:, fc // 2, fc % 2, :],
                                         t[:], n2b[:])

            # =============== Stage A: load x, LN1, paired AllGather ========
            psA_cm = tc.tile_pool(name="psA", bufs=1, space="PSUM")
            psum = psA_cm.__enter__()
            poolA_cm = tc.tile_pool(name="poolA", bufs=1)
            poolA = poolA_cm.__enter__()

            x_bf = poolA.tile([P, FC, TS], BF16)
            nc.sync.dma_start(x_bf[:], xb_d[:])
            nc.sync.dma_start(x_fm[:], x_d[:])
            nc.sync.dma_start(wqk[:], wqk_d[:])
            nc.sync.dma_start(wv[:], wv_d[:])

            aT = poolA.tile([P, FCP, 2, TS], FP8)
            emit_ln(psum, x_bf, aT, "l1")

            agb = poolA.tile([P, FCP, 2, NC_, TS], FP8)
            for p in range(FCP):
                nc.sync.dma_start(
                    ag_in[p][:], aT[:, p, :, :].rearrange("p i t -> i p t"))
                if solo:
                    for i in range(2):
                        nc.sync.dma_start(
                            agb[:, p, i, :, :],
                            ag_in[p][i][None].broadcast_to((NC_, P, TS))
                            .rearrange("s p t -> p s t"))
                else:
                    nc.gpsimd.collective_compute(
                        "AllGather", mybir.AluOpType.bypass,
                        replica_groups=[list(range(NC_))],
                        ins=[ag_in[p].opt()], outs=[ag_out[p].opt()])
                    nc.sync.dma_start(
                        agb[:, p, :, :, :],
                        ag_out[p][:].rearrange("s i p t -> p i s t"))
            nc.sync.dma_start(wproj[:], wproj_d[:])

            # =============== Stage C: qkv (fp8 DoubleRow) ===============
            for cb in range(NC_):
                b = cb // 4
                ps_qk = psum.tile([P, 2, TS], F32, tag="psqk", bufs=2)
                ps_v = psum.tile([P, 4, P], F32, tag="psv", bufs=2)
                for p in range(FCP):
                    st_, sp_ = (p == 0), (p == FCP - 1)
                    nc.tensor.matmul(ps_qk[:, 0, :], wqk[:, p, :, 0:128],
                                     agb[:, p, :, cb, :], start=st_, stop=sp_,
                                     perf_mode=DR)
                    nc.tensor.matmul(ps_qk[:, 1, :], wqk[:, p, :, 128:256],
                                     agb[:, p, :, cb, :], start=st_, stop=sp_,
                                     perf_mode=DR)
                    for st in range(4):
                        nc.tensor.matmul(
                            ps_v[:, st, :],
                            agb[:, p, :, cb, 128 * st:128 * st + 128],
                            wv[:, p, :, :], start=st_, stop=sp_, perf_mode=DR)
                nc.scalar.copy(qkT[:, :, cb, :], ps_qk[:])
                kp0 = 2 * (cb % 4)
                psv4 = ps_v[:].rearrange("p (a b) c -> p a b c", a=2)
                nc.scalar.copy(v_sb[:, b, kp0:kp0 + 2, :, 64:128],
                               psv4[:, :, :, 0:64])
                nc.scalar.copy(v_sb[:, b, kp0:kp0 + 2, :, 192:256],
                               psv4[:, :, :, 64:128])

            poolA_cm.__exit__(None, None, None)
            psA_cm.__exit__(None, None, None)

            # prefetch w1 early (DMA idle after stage A; Pool queue issue)
            w1pool_cm = tc.tile_pool(name="w1pool", bufs=1)
            w1pool = w1pool_cm.__enter__()
            w1_sb = w1pool.tile([P, M1, FCP, 2, P], FP8)
            for i in range(4):
                nc.gpsimd.dma_start(
                    w1_sb[:, 8 * i:8 * i + 8, :, :, :],
                    w1_d[8 * i:8 * i + 8].rearrange("m p f i c -> p m f i c"))

            # =============== Stage D: attention ===============
            psD_cm = tc.tile_pool(name="psD", bufs=1, space="PSUM")
            psum = psD_cm.__enter__()

            def emit_attn_block(h, b, flip):
                hr = slice(64 * h, 64 * h + 64)
                for qt in reversed(range(NQT)):
                    nkc = 4 * qt + 4
                    ps_o = psum.tile([P, TS], F32, tag="pso", bufs=2)
                    for pr in range(nkc // 2):
                        d_e = 2 * pr - 4 * qt
                        ext = 128 * d_e if d_e > 0 else 0
                        ps_s = psum.tile([P, 2, TS], F32, tag="pss", bufs=3)
                        for j in range(2):
                            kc = 2 * pr + j
                            cb_k = 4 * b + kc // 4
                            sl = (kc % 4) * P
                            nc.tensor.matmul(
                                ps_s[:, j, ext:TS],
                                qkT[hr, 1, cb_k, sl:sl + P],
                                qkT[hr, 0, 4 * b + qt, ext:TS],
                                start=True, stop=True)
                        pT = work.tile([P, 2, TS], FP8, tag="pT", bufs=3)
                        nc.scalar.activation(pT[:, :, ext:TS], ps_s[:, :, ext:TS],
                                             Exp, scale=1.0 / (WS * WS))
                        if d_e >= 0:
                            # zero masked region: keep where qf-ext >= kp+128j
                            w_sel = min(384, TS - ext)
                            nc.gpsimd.affine_select(
                                out=pT[:, :, ext:ext + w_sel],
                                in_=pT[:, :, ext:ext + w_sel],
                                compare_op=mybir.AluOpType.is_ge,
                                fill=0.0, base=0,
                                pattern=[[-128, 2], [1, w_sel]],
                                channel_multiplier=-1)
                        nc.tensor.matmul(
                            ps_o[:, ext:TS],
                            v_sb[:, b, pr, :, 128 * h:128 * h + 128],
                            pT[:, :, ext:TS],
                            start=(pr == 0), stop=(pr == nkc // 2 - 1),
                            perf_mode=DR, skip_group_check=True)
                    rec = work.tile([1, TS], F32, tag="rec", bufs=2)
                    nc.vector.reciprocal(rec[:], ps_o[0:1, :])
                    recb = work.tile([P, TS], F32, tag="recb", bufs=2)
                    nc.gpsimd.partition_broadcast(recb[:], rec[:])
                    nc.vector.scalar_tensor_tensor(
                        out=oT[h][64:128, 4 * b + qt, :],
                        in0=ps_o[64:128, :], scalar=1.0 / WS,
                        in1=recb[64:128, :],
                        op0=mybir.AluOpType.mult, op1=mybir.AluOpType.mult)

            def emit_a2a(h):
                nc.sync.dma_start(
                    a2a_in[h][:], oT[h][64:128, :, :].rearrange("p s t -> s p t"))
                if solo:
                    src = a2a_in[h][:]
                else:
                    nc.gpsimd.collective_compute(
                        "AllToAll", mybir.AluOpType.bypass,
                        replica_groups=[list(range(NC_))],
                        ins=[a2a_in[h].opt()], outs=[a2a_out[h].opt()])
                    src = a2a_out[h][:]
                nc.sync.dma_start(
                    attn_fm[64 * h:64 * h + 64, :, :],
                    src.rearrange("s p t -> p s t"))

            emit_attn_block(0, 0, False)
            emit_attn_block(1, 0, True)
            emit_attn_block(0, 1, False)
            emit_a2a(0)
            emit_attn_block(1, 1, True)
            emit_a2a(1)

            psD_cm.__exit__(None, None, None)

            # =============== Stage F: proj + residual + LN2 ===============
            psF_cm = tc.tile_pool(name="psF", bufs=1, space="PSUM")
            psum = psF_cm.__enter__()
            for m in range(FC):
                ps_p = psum.tile([P, TS], F32, tag="psp", bufs=2)
                for sp in range(4):
                    nc.tensor.matmul(ps_p[:], wproj[:, sp, :, m, :],
                                     attn_fm[:, 2 * sp:2 * sp + 2, :],
                                     start=(sp == 0), stop=(sp == 3),
                                     perf_mode=DR)
                nc.vector.scalar_tensor_tensor(
                    out=r1[:, m, :], in0=ps_p[:], scalar=1.0 / WS,
                    in1=x_fm[:, m, :],
                    op0=mybir.AluOpType.mult, op1=mybir.AluOpType.add)
                nc.gpsimd.tensor_copy(r1_bf[:, m, :], r1[:, m, :])

            emit_ln(psum, r1_bf, bT, "l2")
            psF_cm.__exit__(None, None, None)

            # =============== Stage G: FFN (fp8 DoubleRow) ===============
            with tc.tile_pool(name="w2pool", bufs=3) as w2pool, \
                 tc.tile_pool(name="psG", bufs=1, space="PSUM") as psG:
                for j in range(M1P):
                    ps_h = psG.tile([P, 2, TS], F32, tag="psh", bufs=2)
                    for half in range(2):
                        m1 = 2 * j + half
                        for p in range(FCP):
                            nc.tensor.matmul(
                                ps_h[:, half, :], w1_sb[:, m1, p, :, :],
                                bT[:, p, :, :], start=(p == 0),
                                stop=(p == FCP - 1), perf_mode=DR)
                    nc.scalar.activation(hT[:, j, :, :], ps_h[:], Gelu,
                                         scale=1.0 / WS)

                for m2 in range(FC):
                    w2_t = w2pool.tile([P, M1P, 2, P], FP8, tag="w2t")
                    nc.gpsimd.dma_start(w2_t[:], w2_d[m2])
                    ps_f = psG.tile([P, TS], F32, tag="psf", bufs=2)
                    for kp in range(M1P):
                        nc.tensor.matmul(ps_f[:], w2_t[:, kp, :, :],
                                         hT[:, kp, :, :], start=(kp == 0),
                                         stop=(kp == M1P - 1), perf_mode=DR)
                    of = work.tile([P, TS], F32, tag="of", bufs=2)
                    nc.vector.scalar_tensor_tensor(
                        out=of[:], in0=ps_f[:], scalar=1.0 / WS,
                        in1=r1[:, m2, :],
                        op0=mybir.AluOpType.mult, op1=mybir.AluOpType.add)
                    nc.gpsimd.dma_start(out_d[m2], of[:])
            w1pool_cm.__exit__(None, None, None)
    return nc


# ==================== host side ====================

_CACHE = {}


def _build_and_compile():
    if "nc" in _CACHE:
        return _CACHE["nc"]
    import concourse.bass as bass
    import concourse.mybir as mybir
    import concourse.tile as tile
    from concourse import bacc
    nc = bacc.Bacc("TRN2", target_bir_lowering=False, debug=False,
                   num_devices=NC_)
    build(nc, tile, mybir, bass, solo=False)
    nc.compile()
    _CACHE["nc"] = nc
    return nc


def _prep_inputs(x, w_qkv, w_proj, w1, w2, ln1_g, ln1_b, ln2_g, ln2_b):
    f8 = ml_dtypes.float8_e4m3
    bf = ml_dtypes.bfloat16
    x = np.asarray(x, np.float32)
    w_qkv = np.asarray(w_qkv, np.float32)
    w_proj = np.asarray(w_proj, np.float32)
    w1 = np.asarray(w1, np.float32)
    w2 = np.asarray(w2, np.float32)
    ln1_g = np.asarray(ln1_g, np.float32)
    ln2_g = np.asarray(ln2_g, np.float32)
    assert not np.any(np.asarray(ln1_b)) and not np.any(np.asarray(ln2_b)), \
        "nonzero LN bias not wired in this build"

    x_flat = np.ascontiguousarray(x.reshape(B * T, D))
    wq = w_qkv[:, :D] * (SCALE * WS * ln1_g[:, None])
    wk = w_qkv[:, D:2 * D] * (WS * ln1_g[:, None])
    wv_full = w_qkv[:, 2 * D:] * (WS * ln1_g[:, None])
    w1f = w1 * (WS * ln2_g[:, None])
    wproj_s = w_proj * WS
    w2_s = w2 * WS

    # [p, sp, i, m, c]: row 128*(2sp+i)+p, col 128m+c
    wproj_t = np.ascontiguousarray(
        wproj_s.reshape(4, 2, P, FC, P).transpose(2, 0, 1, 3, 4)).astype(f8)
    # [m1, p, fp, i, c]: row 128*(2fp+i)+p, col 128*m1+c
    w1_t = np.ascontiguousarray(
        w1f.reshape(FCP, 2, P, M1, P).transpose(3, 2, 0, 1, 4)).astype(f8)
    # [m2, p, kp, i, c]: row 128*(2kp+i)+p, col 128*m2+c
    w2_t = np.ascontiguousarray(
        w2_s.reshape(M1P, 2, P, FC, P).transpose(3, 2, 0, 1, 4)).astype(f8)

    in_maps = []
    for c in range(NC_):
        hcols = slice(128 * c, 128 * c + 128)
        wqk_c = np.concatenate([wq[:, hcols], wk[:, hcols]], axis=1)  # [D,256]
        wqk_t = np.ascontiguousarray(
            wqk_c.reshape(FCP, 2, P, 256).transpose(2, 0, 1, 3)).astype(f8)
        wv_t = np.ascontiguousarray(
            wv_full[:, hcols].reshape(FCP, 2, P, P).transpose(2, 0, 1, 3)
        ).astype(f8)
        x_c = x_flat[c * TS:(c + 1) * TS]          # [TS, D]
        x_cT = np.ascontiguousarray(
            x_c.T.reshape(FC, P, TS).transpose(1, 0, 2))  # [P, FC, TS]
        in_maps.append({
            "x_sl": x_cT,
            "x_bf": x_cT.astype(bf),
            "wqk": wqk_t,
            "wv": wv_t,
            "wproj": wproj_t,
            "w1": w1_t,
            "w2": w2_t,
        })
    return in_maps


def kernel(x, w_qkv, w_proj, w1, w2, ln1_g, ln1_b, ln2_g, ln2_b):
    from concourse.bass_utils import run_bass_kernel_spmd
    nc = _build_and_compile()
    in_maps = _prep_inputs(x, w_qkv, w_proj, w1, w2,
                           ln1_g, ln1_b, ln2_g, ln2_b)
    res = run_bass_kernel_spmd(nc, in_maps, list(range(NC_)))
    slices = []
    for c in range(NC_):
        o = res.results[c]["out_sl"]            # [FC, P, TS]
        slices.append(o.transpose(2, 0, 1).reshape(TS, D))
    out = np.concatenate(slices, axis=0)
    return np.ascontiguousarray(out.reshape(B, T, D)).astype(np.float32)
